# revision 5
# baseline (speedup 1.0000x reference)
"""KDE mixture evaluation on 8 Trainium2 NeuronCores.

out[m] = sum_n w_n * exp(-||p_m - s_n||^2 / (2 b^2))

Factorization used (exact algebra, no clamp needed for this data regime):
  out[m] = exp(-||p_m||^2/(2b^2)) * S_m
  S_m    = sum_n exp((p_m . s_n + beta_n) / b^2),  beta_n = b^2 ln w_n - ||s_n||^2/2

beta is folded into the matmul contraction as two extra bf16 rows (hi/lo
split), so the device kernel is just: bf16 matmul (K=66) -> ScalarE Exp with
accum_out performing the weighted row reduction in the same instruction.

Sharding: M query rows split across 8 cores (2048 each); spatial/beta
replicated. Host does all O(M+N) pre/post work in numpy.
"""

import sys

for _p in ("/opt/trn_rl_repo",):
    if _p not in sys.path:
        sys.path.append(_p)

import numpy as np
import ml_dtypes

import concourse.bass as bass
import concourse.bacc as bacc
import concourse.tile as tile
from concourse import mybir
from concourse.bass_utils import run_bass_kernel_spmd

BF16 = ml_dtypes.bfloat16

M, N, D = 16384, 16384, 64
NCORES = 8
M_LOC = M // NCORES          # 2048 query rows per core
P = 128                      # partition tile (m rows per matmul output)
MT = M_LOC // P              # 16 m-tiles per core
NCHUNK = 2048                # free-dim width of one ACT instruction (4 PSUM banks)
NJ = N // NCHUNK             # 8 n-chunks
MMW = 512                    # matmul moving free dim (1 PSUM bank, fp32)
QMM = NCHUNK // MMW          # 4 matmuls per chunk
K = D + 2                    # contraction: 64 data dims + beta_hi + beta_lo

_BUILD_CACHE = {}


def _build(scale: float):
    """Emit the per-core Bass kernel. `scale` = 1/b^2 (ACT affine)."""
    nc = bacc.Bacc(None, target_bir_lowering=False)
    pts = nc.declare_dram_parameter("pts", [K, M_LOC], mybir.dt.bfloat16, isOutput=False)
    spat = nc.declare_dram_parameter("spat", [K, N], mybir.dt.bfloat16, isOutput=False)
    acc_d = nc.declare_dram_parameter("acc", [P, MT * NJ], mybir.dt.float32, isOutput=True)

    with tile.TileContext(nc) as tc:
        with (
            tc.tile_pool(name="sb", bufs=1) as sb,
            tc.tile_pool(name="ework", bufs=2) as ework,
            tc.tile_pool(name="ps", bufs=2, space="PSUM") as ps,
        ):
            pts_t = sb.tile([K, M_LOC], mybir.dt.bfloat16, tag="pts")
            nc.sync.dma_start(out=pts_t[:], in_=pts[:])

            spat_tiles = []
            for j in range(NJ):
                st = sb.tile([K, NCHUNK], mybir.dt.bfloat16, tag=f"spat{j}")
                nc.sync.dma_start(out=st[:], in_=spat[:, j * NCHUNK:(j + 1) * NCHUNK])
                spat_tiles.append(st)

            zero_bias = sb.tile([P, 1], mybir.dt.float32, tag="zb")
            nc.vector.memset(zero_bias[:], 0.0)

            acc = sb.tile([P, MT * NJ], mybir.dt.float32, tag="acc")

            for t in range(MT):
                lhsT = pts_t[:, t * P:(t + 1) * P]
                for j in range(NJ):
                    pt = ps.tile([P, NCHUNK], mybir.dt.float32, tag="ps")
                    for q in range(QMM):
                        nc.tensor.matmul(
                            pt[:, q * MMW:(q + 1) * MMW],
                            lhsT=lhsT,
                            rhs=spat_tiles[j][:, q * MMW:(q + 1) * MMW],
                            start=True,
                            stop=True,
                        )
                    et = ework.tile([P, NCHUNK], mybir.dt.float32, tag="e")
                    nc.scalar.activation(
                        et[:],
                        pt[:],
                        mybir.ActivationFunctionType.Exp,
                        bias=zero_bias[:],
                        scale=scale,
                    )
                    col = t * NJ + j
                    nc.vector.tensor_reduce(
                        acc[:, col:col + 1],
                        et[:],
                        axis=mybir.AxisListType.X,
                        op=mybir.AluOpType.add,
                    )

            nc.sync.dma_start(out=acc_d[:], in_=acc[:])
    nc.compile()
    return nc


def _get_nc(scale: float):
    key = float(np.float32(scale))
    if key not in _BUILD_CACHE:
        _BUILD_CACHE[key] = _build(key)
    return _BUILD_CACHE[key]


def _prep_inputs(points, spatial_data, weights, bandwidth):
    points = np.asarray(points, dtype=np.float32)
    spatial = np.asarray(spatial_data, dtype=np.float32)
    weights = np.asarray(weights, dtype=np.float32)
    b = float(np.asarray(bandwidth, dtype=np.float32))
    b2 = b * b

    s_sq = (spatial.astype(np.float64) ** 2).sum(axis=1)            # [N]
    w64 = np.maximum(weights.astype(np.float64), 1e-300)
    beta = b2 * np.log(w64) - 0.5 * s_sq                            # [N]
    beta_hi = beta.astype(np.float32).astype(BF16)
    beta_lo = (beta - beta_hi.astype(np.float64)).astype(np.float32).astype(BF16)

    spat_aug = np.empty((K, N), dtype=BF16)
    spat_aug[:D] = spatial.T.astype(BF16)
    spat_aug[D] = beta_hi
    spat_aug[D + 1] = beta_lo

    in_maps = []
    for c in range(NCORES):
        pl = points[c * M_LOC:(c + 1) * M_LOC]
        pts_aug = np.empty((K, M_LOC), dtype=BF16)
        pts_aug[:D] = pl.T.astype(BF16)
        pts_aug[D:] = BF16(1.0)
        in_maps.append({"pts": pts_aug, "spat": spat_aug})

    p_sq = (points.astype(np.float64) ** 2).sum(axis=1)             # [M]
    prefac = np.exp(-p_sq / (2.0 * b2))                             # [M] float64
    return in_maps, prefac, 1.0 / b2


def _postprocess(results, prefac):
    out = np.empty(M, dtype=np.float32)
    for c in range(NCORES):
        S = results[c]["acc"].astype(np.float64)                    # [P, MT*NJ]
        Ssum = S.reshape(P, MT, NJ).sum(axis=2)                     # [p, t]
        block = Ssum.T.reshape(M_LOC)                               # m = t*P + p
        out[c * M_LOC:(c + 1) * M_LOC] = (
            prefac[c * M_LOC:(c + 1) * M_LOC] * block
        ).astype(np.float32)
    return out


def _run(points, spatial_data, weights, bandwidth, trace=False, **spmd_kwargs):
    in_maps, prefac, scale = _prep_inputs(points, spatial_data, weights, bandwidth)
    nc = _get_nc(scale)
    res = run_bass_kernel_spmd(
        nc, in_maps, list(range(NCORES)), trace=trace, **spmd_kwargs
    )
    return _postprocess(res.results, prefac), res


def kernel(points, spatial_data, weights, bandwidth):
    out, _ = _run(points, spatial_data, weights, bandwidth)
    return out
